# revision 6
# baseline (speedup 1.0000x reference)
"""ArcticMoE top-2 MoE kernel for 8 Trainium2 NeuronCores.

Strategy (expert-parallel, sparse):
  - Host: router (logits -> softmax -> top-k -> renorm), per-expert token
    gather, transpose activations/weights into PE-friendly layouts.
  - Device (SPMD, 8 cores, 2 expert slots/core): for each owned expert compute
    y.T = w2 @ (silu(g.T) * u.T) where [g.T; u.T] = w13 @ x_e.T, bf16 matmuls,
    feature dim on partitions, tokens on the moving/free axis.
  - Host: unweighted expert outputs scatter-added back with routing coefs.

The reference computes every expert densely; only top-2 contribute, so the
sparse form does 1/8th the FLOPs. Experts are assigned to slots by load
(8 biggest -> slot0 with capacity C0, 8 smallest -> slot1 with C1) so the
SPMD graph pads as little as possible. No collectives: each core's work is
independent and the combine happens on host.

PE efficiency: the k-loop loads each weight k-slice once and issues the
matmuls for every token block back-to-back; the duplicate Ldweights bass
emits for the second block are stripped from the BIR before compile.
"""

import numpy as np

T, H, I, E = 4096, 2048, 2048, 16
N_CORES = 8
EPC = E // N_CORES   # expert slots per core
KT = H // 128        # k-tiles over H (matmul 1 contraction)
MT1 = 2 * I // 128   # m-tiles over 2I (matmul 1 output rows)
IT = I // 128        # k-tiles over I (matmul 2 contraction)
MT2 = H // 128       # m-tiles over H (matmul 2 output rows)

_CACHE = {}
LAST_EXEC_NS = None  # exec_time_ns from the last run, when profiling is available


def _pad(v, g):
    return max(g, -(-v // g) * g)


def _blocks_of(C):
    return [C] if C <= 512 else [C // 2, C // 2]


def _dedup_ldweights(nc):
    """Remove InstLdweights that reload the identical weights AP as the
    previous Ldweights in the same basic block with only Matmults between."""
    removed = 0
    for bb in nc.m.functions[0].blocks:
        insts = bb.instructions
        keep = []
        last_key = None
        for inst in insts:
            tn = type(inst).__name__
            if tn == "InstLdweights":
                key = (
                    str(inst.ins[0]), str(inst.tile_position),
                    str(inst.tile_size), str(inst.perf_mode),
                    bool(inst.is_transpose),
                )
                if key == last_key and not inst.has_wait() and not inst.has_update():
                    removed += 1
                    continue
                last_key = key
            elif tn == "InstMatmult":
                pass
            else:
                last_key = None
            keep.append(inst)
        if len(keep) != len(insts):
            bb.instructions = keep
    return removed


def _build(caps):
    """Build + compile the per-core Bass program for slot capacities `caps`."""
    import concourse.mybir as mybir
    from concourse import bacc
    from concourse.tile import TileContext

    F32 = mybir.dt.float32
    BF16 = mybir.dt.bfloat16
    Silu = mybir.ActivationFunctionType.Silu

    nc = bacc.Bacc("TRN2", target_bir_lowering=False, debug=False, num_devices=N_CORES)
    xts_d, outs_d = [], []
    for s, C in enumerate(caps):
        xts_d.append(nc.dram_tensor(f"xt{s}", [128, KT, C], BF16, kind="ExternalInput"))
        outs_d.append(nc.dram_tensor(f"out{s}", [MT2, 128, C], F32, kind="ExternalOutput"))
    w1 = nc.dram_tensor("w1", [EPC, MT1, 128, KT, 128], BF16, kind="ExternalInput")
    w2 = nc.dram_tensor("w2", [EPC, MT2, 128, IT, 128], BF16, kind="ExternalInput")

    Cmax = max(caps)
    with TileContext(nc) as tc:
        with (
            tc.tile_pool(name="x", bufs=2) as xpool,
            tc.tile_pool(name="h", bufs=2) as hpool,
            tc.tile_pool(name="w", bufs=6) as wpool,
            tc.tile_pool(name="y", bufs=3) as ypool,
            tc.tile_pool(name="ps", bufs=8, space="PSUM") as pspool,
        ):
            def mm_group(dst_tiles, wtile, src, bss, KTN):
                for k in range(KTN):
                    for ps, bs in zip(dst_tiles, bss):
                        nc.tensor.matmul(
                            ps[:], wtile[:, k], src[:, k, bs],
                            start=(k == 0), stop=(k == KTN - 1),
                        )

            for s, C in enumerate(caps):
                bls = _blocks_of(C)
                bss = []
                off = 0
                for CB in bls:
                    bss.append(slice(off, off + CB))
                    off += CB
                xts = xpool.tile([128, KT, Cmax], BF16, tag="x", name="xts")[:, :, :C]
                nc.sync.dma_start(xts[:], xts_d[s].ap())
                hbuf = hpool.tile([128, IT, Cmax], BF16, tag="h", name="hbuf")[:, :, :C]
                # ---- matmul 1 (w13 @ x.T) fused with SiluAndMul ----
                for j in range(IT):
                    wg = wpool.tile([128, KT, 128], BF16, tag="w")
                    nc.sync.dma_start(wg[:], w1[s, j])
                    wu = wpool.tile([128, KT, 128], BF16, tag="w")
                    nc.sync.dma_start(wu[:], w1[s, j + IT])
                    psgs = [
                        pspool.tile([128, 512], F32, tag="ps", name="psg")[:, :CB]
                        for CB in bls
                    ]
                    mm_group(psgs, wg, xts, bss, KT)
                    psus = [
                        pspool.tile([128, 512], F32, tag="ps", name="psu")[:, :CB]
                        for CB in bls
                    ]
                    mm_group(psus, wu, xts, bss, KT)
                    for b in range(len(bls)):
                        hs = hbuf[:, j, bss[b]]
                        nc.scalar.activation(hs, psgs[b][:], Silu)
                        nc.vector.tensor_mul(hs, hs, psus[b][:])
                # ---- matmul 2 (w2 @ h.T) ----
                for m in range(MT2):
                    w2t = wpool.tile([128, IT, 128], BF16, tag="w")
                    nc.sync.dma_start(w2t[:], w2[s, m])
                    yt = ypool.tile([128, Cmax], F32, tag="y", name="yt")[:, :C]
                    pss = [
                        pspool.tile([128, 512], F32, tag="ps", name="pso")[:, :CB]
                        for CB in bls
                    ]
                    mm_group(pss, w2t, hbuf, bss, IT)
                    for b in range(len(bls)):
                        nc.vector.tensor_copy(yt[:, bss[b]], pss[b][:])
                    nc.sync.dma_start(outs_d[s][m], yt[:])
    _dedup_ldweights(nc)
    nc.compile()
    return nc


def _get_nc(caps):
    if caps not in _CACHE:
        _CACHE[caps] = _build(caps)
    return _CACHE[caps]


def _route(x, gate_w, top_k):
    """Replicate the reference router on host. Returns (order [T,k], coefs [T,k])."""
    logits = x @ gate_w.T
    m = logits.max(-1, keepdims=True)
    ex = np.exp(logits - m)
    scores = ex / ex.sum(-1, keepdims=True)
    order = np.argsort(-scores, axis=-1)[:, :top_k]
    tw = np.take_along_axis(scores, order, -1)
    if top_k > 1:
        tw = tw / tw.sum(-1, keepdims=True)
    return order.astype(np.int64), tw.astype(np.float32)


def kernel(x, gate_w, ws, w2s, top_k):
    import ml_dtypes
    from concourse.bass_utils import run_bass_kernel_spmd

    bf16 = ml_dtypes.bfloat16
    x = np.ascontiguousarray(np.asarray(x, dtype=np.float32))
    gate_w = np.asarray(gate_w, dtype=np.float32)
    ws = np.asarray(ws, dtype=np.float32)
    w2s = np.asarray(w2s, dtype=np.float32)
    top_k = int(np.asarray(top_k))

    order, tw = _route(x, gate_w, top_k)

    ids = [np.nonzero((order == e).any(-1))[0] for e in range(E)]
    counts = np.array([len(i) for i in ids])

    # slot assignment: 8 biggest experts -> slot0, 8 smallest -> slot1
    perm = np.argsort(-counts, kind="stable")
    slot_experts = [perm[:N_CORES], perm[N_CORES:]]
    caps = tuple(
        _pad(int(counts[se].max()), 64 if counts[se].max() > 512 else 32)
        for se in slot_experts
    )
    assert caps[0] <= 1024 and caps[1] <= 1024, caps

    nc = _get_nc(caps)

    # weights, pre-transposed + tiled + bf16, per expert
    # w13 lhsT tile layout: w1h[e, m, p, k, c] = ws[e, m*128+c, k*128+p]
    in_maps = []
    for c in range(N_CORES):
        owned = [int(slot_experts[s][c]) for s in range(EPC)]
        m = {
            "w1": np.ascontiguousarray(
                ws[owned].reshape(EPC, MT1, 128, KT, 128).transpose(0, 1, 4, 3, 2)
            ).astype(bf16),
            "w2": np.ascontiguousarray(
                w2s[owned].reshape(EPC, MT2, 128, IT, 128).transpose(0, 1, 4, 3, 2)
            ).astype(bf16),
        }
        for s, e in enumerate(owned):
            C = caps[s]
            xt_host = np.zeros((128, KT, C), bf16)
            xe = x[ids[e]]  # [n_e, H]
            # xt[p, k, t] = xe[t, k*128+p]
            xt_host[:, :, : len(ids[e])] = (
                xe.T.reshape(KT, 128, -1).transpose(1, 0, 2).astype(bf16)
            )
            m[f"xt{s}"] = xt_host
        in_maps.append(m)

    try:
        res = run_bass_kernel_spmd(nc, in_maps, core_ids=list(range(N_CORES)))
    except ModuleNotFoundError:
        # BASS_TRACE set but this axon client has no NTFF profile hook
        import os

        os.environ["BASS_NEVER_TRACE"] = "1"
        res = run_bass_kernel_spmd(nc, in_maps, core_ids=list(range(N_CORES)))
    global LAST_EXEC_NS
    LAST_EXEC_NS = res.exec_time_ns

    out = np.zeros((T, H), np.float32)
    for c in range(N_CORES):
        for s in range(EPC):
            e = int(slot_experts[s][c])
            n_e = len(ids[e])
            if n_e == 0:
                continue
            yts = res.results[c][f"out{s}"]  # [MT2, 128, C]
            ye = yts.transpose(2, 0, 1).reshape(caps[s], H)[:n_e]
            sel = order[ids[e]] == e  # [n_e, k]
            coef = (tw[ids[e]] * sel).sum(-1).astype(np.float32)
            out[ids[e]] += coef[:, None] * ye
    return out
